# revision 7
# baseline (speedup 1.0000x reference)
"""NonLocalAttention Trainium2 kernel.

Reference computation (N=2, C=64, CR=32, H=W=96, HW=9216):
    e1  = PReLU(w1 @ inputa + b1)   # [N,32,HW]   (queries)
    e2  = PReLU(w2 @ inputb + b2)   # [N,32,HW]   (keys)
    asm = PReLU(wa @ inputa + ba)   # [N,64,HW]   (values)
    out = softmax(e1^T e2, axis=keys) @ asm^T + inputa

Sharding: 8 cores = 2 batches x 4 query-chunks of 2304 rows. Each core gets
its batch's full inputa/inputb (for keys/values) plus its query chunk, and
writes a disjoint [64, 2304] slice of the output. No collectives.

Per-core kernel (flash-style, never materializes [HW,HW]):
  - conv biases are folded into the matmuls by augmenting the contraction
    dim with a ones-row (inputs shipped as [65, HW]) and appending the bias
    row to the transposed weights.
  - PReLU slope is exactly 0.25 (power of two), so prelu(x) == max(x, .25x)
    exactly in fp32; two DVE ops (walrus allows one PSUM operand per op).
  - attention uses the S^T = e2^T e1 orientation: keys land on the PSUM
    partition dim, so the PV matmul needs no transposes at all, and an
    all-ones 65th column in the value tiles makes the PV matmul emit the
    softmax denominator as PSUM row 64 for free.
  - scores are bounded (|s| <= 32 * max|e1| * max|e2| << 88) so exp needs
    no max-subtraction; softmax normalization divides at the end.
  - big matmuls run as float32r (1 PE cycle/row at N>=256 vs 4 for fp32);
    fp32r-consumed tiles are declared float32r end-to-end (bir verifier
    requires producers to be fp32r-typed). float32r stores plain fp32 bits,
    so DVE reads of those tiles (e.g. the residual add) stay exact.
"""

import numpy as np

C = 64
CP = C + 1  # augmented contraction (ones row folds the bias add in)
CR = 32
HW = 9216
QCH = 2304  # query rows per core
NKT = HW // 128  # 72 key tiles
NCORES = 8
QBLOCKS = [(0, 512), (512, 512), (1024, 512), (1536, 512), (2048, 256)]


def _ensure_ntff_hook():
    """Best-effort registration of the axon NTFF profile hook; the agent
    image's antenv package lacks axon_hooks, which would make any traced
    run crash on import instead of degrading."""
    import sys
    import types

    try:
        import antenv.axon_hooks  # noqa: F401

        return
    except ImportError:
        pass
    try:
        import antenv
        from trn_agent_boot.trn_boot import _ntff_profile_via_ctypes

        hook = _ntff_profile_via_ctypes("/opt/axon/libaxon_pjrt.so")
        mod = types.ModuleType("antenv.axon_hooks")
        _h = [hook]
        mod.get_axon_ntff_profile_hook = lambda: _h[0]
        mod.set_axon_ntff_profile_hook = lambda h: _h.__setitem__(0, h)
        sys.modules["antenv.axon_hooks"] = mod
        antenv.axon_hooks = mod
    except Exception:
        pass


def build_program(a1: float, a2: float, aa: float):
    import concourse.bacc as bacc
    import concourse.tile as tile
    from concourse import mybir

    f32 = mybir.dt.float32
    f32r = mybir.dt.float32r
    AF = mybir.ActivationFunctionType

    nc = bacc.Bacc()
    xa = nc.dram_tensor("xa", [CP, HW], f32r, kind="ExternalInput")
    xb = nc.dram_tensor("xb", [CP, HW], f32r, kind="ExternalInput")
    xq = nc.dram_tensor("xq", [CP, QCH], f32r, kind="ExternalInput")
    w1t = nc.dram_tensor("w1t", [CP, CR], f32r, kind="ExternalInput")
    w2t = nc.dram_tensor("w2t", [CP, CR], f32r, kind="ExternalInput")
    wat = nc.dram_tensor("wat", [CP, C], f32r, kind="ExternalInput")
    ones_c = nc.dram_tensor("ones_c", [128, NKT], f32r, kind="ExternalInput")
    out = nc.dram_tensor("out", [C, QCH], f32, kind="ExternalOutput")

    with tile.TileContext(nc) as tc:
        with (
            tc.tile_pool(name="consts", bufs=1) as consts,
            tc.tile_pool(name="big", bufs=1) as big,
            tc.tile_pool(name="ps", bufs=2, space="PSUM") as ps,
            tc.tile_pool(name="po", bufs=1, space="PSUM") as ps_o,
            tc.tile_pool(name="pb", bufs=1, space="PSUM") as ps_b,
            tc.tile_pool(name="pt", bufs=3) as ptile,
            tc.tile_pool(name="work", bufs=2) as work,
        ):
            # --- constants / weights -------------------------------------
            w1_sb = consts.tile([CP, CR], f32r, tag="w1")
            nc.sync.dma_start(w1_sb[:], w1t[:])
            w2_sb = consts.tile([CP, CR], f32r, tag="w2")
            nc.sync.dma_start(w2_sb[:], w2t[:])
            wa_sb = consts.tile([CP, C], f32r, tag="wa")
            nc.sync.dma_start(wa_sb[:], wat[:])
            ones_sb = consts.tile([1, C], f32r, tag="ones")
            nc.sync.dma_start(ones_sb[:], ones_c[0:1, 0:C])

            # --- activations in, chunked for DMA/compute overlap ---------
            xa_sb = big.tile([CP, HW], f32r, tag="xa")
            xb_sb = big.tile([CP, HW], f32r, tag="xb")
            xq_sb = big.tile([CP, QCH], f32r, tag="xq")
            for off in range(0, HW, QCH):
                nc.sync.dma_start(xa_sb[:, off : off + QCH], xa[:, off : off + QCH])
                nc.sync.dma_start(xb_sb[:, off : off + QCH], xb[:, off : off + QCH])
            nc.sync.dma_start(xq_sb[:], xq[:])

            # --- e1 = prelu(w1 @ xq + b1): [CR, QCH] ---------------------
            e1_sb = big.tile([CR, QCH], f32r, tag="e1")
            for off, nq in QBLOCKS:
                pse = ps.tile([CR, nq], f32, tag="ps")
                nc.tensor.matmul(
                    pse[:], w1_sb[:], xq_sb[:, off : off + nq],
                    start=True, stop=True,
                )
                ya = work.tile([CR, nq], f32, tag="ya1")
                nc.vector.tensor_scalar_mul(ya[:], pse[:], a1)
                nc.vector.tensor_max(e1_sb[:, off : off + nq], ya[:], pse[:])

            # --- e2 = prelu(w2 @ xb + b2): [CR, HW] ----------------------
            e2_sb = big.tile([CR, HW], f32r, tag="e2")
            for off in range(0, HW, 512):
                pse = ps.tile([CR, 512], f32, tag="ps")
                nc.tensor.matmul(
                    pse[:], w2_sb[:], xb_sb[:, off : off + 512],
                    start=True, stop=True,
                )
                ya = work.tile([CR, 512], f32, tag="ya2")
                nc.vector.tensor_scalar_mul(ya[:], pse[:], a2)
                nc.vector.tensor_max(e2_sb[:, off : off + 512], ya[:], pse[:])

            # --- v_aug tiles: [128, 65] per key tile, col 64 = ones ------
            # v = asm^T computed directly transposed: per key tile i,
            # psum[128,64] = xa_aug[:, i*128:(i+1)*128]^T @ wat_aug.
            v_all = big.tile([128, NKT * 65], f32r, tag="vall")
            v3 = v_all[:].rearrange("p (t c) -> p t c", c=65)
            nc.sync.dma_start(v3[:, :, 64], ones_c[:])
            for grp in range(NKT // 8):  # 8 key tiles per psum bank batch
                psv = ps.tile([128, 512], f32, tag="ps")
                for j in range(8):
                    i = grp * 8 + j
                    nc.tensor.matmul(
                        psv[:, j * 64 : (j + 1) * 64],
                        xa_sb[:, i * 128 : (i + 1) * 128],
                        wa_sb[:],
                        start=(j == 0), stop=(j == 7),
                    )
                psv3 = psv[:].rearrange("p (t c) -> p t c", c=64)
                yv = work.tile([128, 512], f32, tag="yv")
                yv3 = yv[:].rearrange("p (t c) -> p t c", c=64)
                nc.vector.tensor_scalar_mul(yv[:], psv[:], aa)
                nc.vector.tensor_max(
                    v3[:, grp * 8 : (grp + 1) * 8, 0:64], yv3[:], psv3[:]
                )

            # --- attention: per q-block, loop key tiles ------------------
            # S^T psum batches 3 key tiles (3 banks) per exp op.
            for off, nq in QBLOCKS:
                kt_per_ps = 1536 // nq  # 3 at nq=512, 6 at nq=256
                po = ps_o.tile([CP, nq], f32, tag="po")
                for g in range(NKT // kt_per_ps):
                    pss = ps.tile([128, 1536], f32, tag="ps")
                    for j in range(kt_per_ps):
                        i = g * kt_per_ps + j
                        colb = j * nq * 4  # byte offset of this matmul
                        nc.tensor.matmul(
                            pss[:, j * nq : (j + 1) * nq],
                            e2_sb[:, i * 128 : (i + 1) * 128],
                            e1_sb[:, off : off + nq],
                            start=(colb % 2048 == 0),
                            stop=((colb + nq * 4) % 2048 == 0),
                        )
                    pt = ptile.tile([128, 1536], f32r, tag="pt")
                    nc.scalar.activation(pt[:], pss[:], AF.Exp)
                    for j in range(kt_per_ps):
                        i = g * kt_per_ps + j
                        nc.tensor.matmul(
                            po[:],
                            v_all[:, i * 65 : (i + 1) * 65],
                            pt[:, j * nq : (j + 1) * nq],
                            start=(i == 0), stop=(i == NKT - 1),
                        )
                # epilogue: out = po[0:64] / po[64] + xq
                rec = work.tile([1, nq], f32r, tag="rec")
                # f32r out only tags the tile for the broadcast matmul; the
                # DVE still computes and stores full-precision fp32 bits.
                with nc.allow_low_precision("f32r tag for broadcast matmul"):
                    nc.vector.reciprocal(rec[:], po[C : C + 1, :])
                pb = ps_b.tile([C, nq], f32, tag="pb")
                nc.tensor.matmul(
                    pb[:], ones_sb[:], rec[:], start=True, stop=True
                )
                rb = work.tile([C, nq], f32, tag="rb")
                nc.vector.tensor_copy(rb[:], pb[:])
                osb = work.tile([C, nq], f32, tag="osb")
                nc.vector.tensor_mul(osb[:], rb[:], po[0:C, :])
                nc.vector.tensor_add(osb[:], osb[:], xq_sb[0:C, off : off + nq])
                nc.sync.dma_start(out[:, off : off + nq], osb[:])
    nc.finalize()
    return nc


def run(inputs: dict, trace: bool = False, tmpdir: str | None = None):
    """Build, compile and run on 8 cores; returns (output, BassKernelResults)."""
    _ensure_ntff_hook()
    from concourse.bass_utils import run_bass_kernel_spmd

    inputa = np.asarray(inputs["inputa"], dtype=np.float32)
    inputb = np.asarray(inputs["inputb"], dtype=np.float32)
    w1 = np.asarray(inputs["w1"], dtype=np.float32)
    b1 = np.asarray(inputs["b1"], dtype=np.float32)
    w2 = np.asarray(inputs["w2"], dtype=np.float32)
    b2 = np.asarray(inputs["b2"], dtype=np.float32)
    wa = np.asarray(inputs["wa"], dtype=np.float32)
    ba = np.asarray(inputs["ba"], dtype=np.float32)
    a1 = float(np.asarray(inputs["a1"]).reshape(-1)[0])
    a2 = float(np.asarray(inputs["a2"]).reshape(-1)[0])
    aa = float(np.asarray(inputs["aa"]).reshape(-1)[0])

    N, Cc, H, W = inputa.shape
    assert (N, Cc, H * W) == (2, C, HW), inputa.shape
    chunks_per_batch = NCORES // N  # 4

    xa_n = inputa.reshape(N, C, HW)
    xb_n = inputb.reshape(N, C, HW)
    ones = np.ones((1, HW), np.float32)

    w1t_aug = np.ascontiguousarray(np.vstack([w1.T, b1[None, :]]), np.float32)
    w2t_aug = np.ascontiguousarray(np.vstack([w2.T, b2[None, :]]), np.float32)
    wat_aug = np.ascontiguousarray(np.vstack([wa.T, ba[None, :]]), np.float32)

    in_maps = []
    for core in range(NCORES):
        b, chunk = divmod(core, chunks_per_batch)
        xa_aug = np.ascontiguousarray(np.vstack([xa_n[b], ones]))
        xb_aug = np.ascontiguousarray(np.vstack([xb_n[b], ones]))
        xq_aug = np.ascontiguousarray(
            xa_aug[:, chunk * QCH : (chunk + 1) * QCH]
        )
        in_maps.append(
            {
                "xa": xa_aug,
                "xb": xb_aug,
                "xq": xq_aug,
                "w1t": w1t_aug,
                "w2t": w2t_aug,
                "wat": wat_aug,
                "ones_c": np.ones((128, NKT), np.float32),
            }
        )

    nc = build_program(a1, a2, aa)
    res = run_bass_kernel_spmd(
        nc, in_maps, list(range(NCORES)), trace=trace, tmpdir=tmpdir
    )

    out = np.empty((N, C, HW), np.float32)
    for core in range(NCORES):
        b, chunk = divmod(core, chunks_per_batch)
        out[b, :, chunk * QCH : (chunk + 1) * QCH] = res.results[core]["out"]
    return out.reshape(N, C, H, W), res


def kernel(**inputs) -> np.ndarray:
    out, _ = run(inputs, trace=False)
    return out
